# revision 23
# baseline (speedup 1.0000x reference)
"""Trainium2 Bass kernel for nn_GCNLearnableModel (3-type heterograph GCN, 9 relations,
3 layers) on 8 NeuronCores.

Strategy (graph/data parallel, one SPMD NEFF):
 - Nodes of each type sharded 8 ways (6250/core). Each core owns the incoming edges
   of its dst shard.
 - Per (layer, relation): every core builds its shard of the "message table"
   T_r = (h[src_type] * out_norm_r) @ W_(l,r)  (node-major, bf16, rows padded to 256B),
   then an AllGather replicates the full 50000-row table to every core's DRAM.
 - Each core gathers its in-edges' source rows with dma_gather (int16 idx; table split
   in two halves to cover 50000 rows), multiplies with streamed one-hot fp8 S-matrices
   on the TensorEngine to segment-sum into 128-node PSUM windows, scales by in_norm and
   accumulates into SBUF acc.
 - LayerNorm+ReLU node-major on the shard; final per-type classifier.
 - Host does index/edge-plan preprocessing only (sort edges, pad windows uniformly
   across cores so a single program fits all 8 cores).
"""
import numpy as np
import ml_dtypes

import concourse.bass as bass
import concourse.bacc as bacc
import concourse.mybir as mybir
import concourse.tile as tile
from concourse.bass_utils import run_bass_kernel_spmd
from concourse.masks import make_identity


def _make_runner(nc, n_cores):
    """jit-once PJRT runner with reusable device inputs (mirrors run_bass_via_pjrt)."""
    import jax
    from jax.sharding import Mesh, PartitionSpec
    from jax.experimental.shard_map import shard_map
    from concourse import bass2jax, mybir as mb

    bass2jax.install_neuronx_cc_hook()
    in_names, out_names, out_avals = [], [], []
    pname = nc.partition_id_tensor.name if nc.partition_id_tensor else None
    for alloc in nc.m.functions[0].allocations:
        if not isinstance(alloc, mb.MemoryLocationSet):
            continue
        name = alloc.memorylocations[0].name
        if alloc.kind == "ExternalInput":
            if name != pname:
                in_names.append(name)
        elif alloc.kind == "ExternalOutput":
            out_names.append(name)
            out_avals.append(jax.core.ShapedArray(tuple(alloc.tensor_shape),
                                                  mb.dt.np(alloc.dtype)))
    n_params = len(in_names)
    all_names = in_names + out_names + ([pname] if pname else [])

    def _body(*args):
        operands = list(args)
        if pname:
            operands.append(bass2jax.partition_id_tensor())
        return tuple(bass2jax._bass_exec_p.bind(
            *operands, out_avals=tuple(out_avals), in_names=tuple(all_names),
            out_names=tuple(out_names), lowering_input_output_aliases=(),
            sim_require_finite=True, sim_require_nnan=True, nc=nc))

    devices = jax.devices()[:n_cores]
    mesh = Mesh(np.asarray(devices), ("core",))
    nin = n_params + len(out_names)
    fn = jax.jit(
        shard_map(_body, mesh=mesh, in_specs=(PartitionSpec("core"),) * nin,
                  out_specs=(PartitionSpec("core"),) * len(out_names),
                  check_rep=False),
        keep_unused=True)
    sharding = jax.sharding.NamedSharding(mesh, PartitionSpec("core"))
    return fn, in_names, out_names, out_avals, sharding

# problem constants (hardcoded per harness contract)
REL = [(0, 1), (2, 1), (2, 0), (0, 0), (1, 2), (1, 0), (0, 0), (1, 1), (2, 2)]
N, IN_F, EMB, HID, OUT, NREL, NLAYERS, E = 50000, 128, 64, 64, 8, 9, 3, 800000
EPS = 1e-5
NC = 8                     # cores
SH = N // NC               # shard size 6250
W49 = 49                   # windows of 128 nodes per shard (49*128 = 6272 >= 6250)
HALF = 32768               # int16 gather base split
CHUNK = 1024               # edges per dma_gather call
BPC = CHUNK // 128         # blocks per chunk (8)

F32, BF16, FP8, I16 = mybir.dt.float32, mybir.dt.bfloat16, mybir.dt.float8e4, mybir.dt.int16
NP_BF16 = np.dtype(mybir.dt.np(BF16))
NP_FP8 = np.dtype(mybir.dt.np(FP8))

# dst-type -> relations, processing order rotates per layer
RELS_OF_T = [[r for r in range(NREL) if REL[r][1] == t] for t in range(3)]


# --------------------------------------------------------------------------- host plan
def _plan(src, dst):
    """Build the uniform (cross-core) edge plan.

    Returns dict with per-(r,half): nbw[w] block counts, nchunk; and per-core
    idx arrays + S arrays; plus norms.
    """
    ones = np.ones(E, np.float32)
    onorm = np.empty((NREL, N), np.float32)
    inorm = np.empty((NREL, N), np.float32)
    for r in range(NREL):
        od = np.bincount(src[r], minlength=N).astype(np.float32)
        idg = np.bincount(dst[r], minlength=N).astype(np.float32)
        onorm[r] = np.maximum(od, 1.0) ** -0.5
        inorm[r] = np.maximum(idg, 1.0) ** -0.5

    # per (r, half, core): edge lists sorted by local dst
    # edge slot layout per vr: windows in order; window w occupies nbw[w]*128 slots
    per_core = [{} for _ in range(NC)]
    nbw_all = {}
    nchunk = {}
    for r in range(NREL):
        d_core = dst[r] // SH
        dloc = dst[r] - d_core * SH
        if True:
            h = 0
            # per-core, per-window counts (single stream; idx = src>>1)
            counts = np.zeros((NC, W49), np.int64)
            core_data = []
            for d in range(NC):
                m = d_core == d
                es, ed = src[r][m], dloc[m]
                o = np.argsort(ed, kind="stable")
                es, ed = es[o], ed[o]
                w = ed >> 7
                counts[d] = np.bincount(w, minlength=W49)
                core_data.append((es, ed))
            nbw = np.maximum(1, -(-counts.max(axis=0) // 128))  # ceil, min 1
            nbw_all[(r, h)] = nbw
            nslot = int(nbw.sum()) * 128
            nck = -(-nslot // CHUNK)
            nchunk[(r, h)] = nck
            nslot_pad = nck * CHUNK
            # window start slot offsets
            wstart = np.concatenate([[0], np.cumsum(nbw) * 128])
            for d in range(NC):
                es, ed = core_data[d]
                w = ed >> 7
                in_w_rank = np.arange(len(ed)) - np.concatenate(
                    [[0], np.cumsum(np.bincount(w, minlength=W49))])[w]
                slot = wstart[w] + in_w_rank
                idx16 = np.zeros(nslot_pad, np.int16)
                idx16[slot] = (es >> 1).astype(np.int16)
                # S: one-hot fp8, [128, nslot_pad*2]: block b cols [256b,256b+128)
                # = even-src, [256b+128, 256b+256) = odd-src
                s_arr = np.zeros((128, nslot_pad * 2), NP_FP8)
                blk = slot >> 7
                s_arr[slot & 127, blk * 256 + (es & 1) * 128 + (ed & 127)] = 1.0
                lanes = idx16.reshape(-1, 16).T
                idx_rep = np.tile(lanes, (8, 1))
                per_core[d][(r, h)] = (idx_rep, s_arr)
    return onorm, inorm, nbw_all, nchunk, per_core


# --------------------------------------------------------------------------- builder
def _build(nbw_all, nchunk):
    nc = bacc.Bacc("TRN2", target_bir_lowering=False, debug=False, num_devices=NC)

    # ---- dram I/O
    feat_fm = nc.dram_tensor("feat_fm", [3, IN_F, 128 * W49], F32, kind="ExternalInput")
    embW = nc.dram_tensor("embW", [3, IN_F, EMB], F32, kind="ExternalInput")
    embB = nc.dram_tensor("embB", [128, 3, EMB], F32, kind="ExternalInput")
    convW = nc.dram_tensor("convW", [NLAYERS, NREL, EMB, HID], F32, kind="ExternalInput")
    biasum = nc.dram_tensor("biasum", [128, NLAYERS * 3, HID], F32, kind="ExternalInput")
    lng = nc.dram_tensor("lng", [128, 3, HID], F32, kind="ExternalInput")
    lnb = nc.dram_tensor("lnb", [128, 3, HID], F32, kind="ExternalInput")
    clsW = nc.dram_tensor("clsW", [3, HID, OUT], F32, kind="ExternalInput")
    clsb = nc.dram_tensor("clsb", [128, 3, OUT], F32, kind="ExternalInput")
    onorm_win = nc.dram_tensor("onorm_win", [NREL, 128, W49], F32, kind="ExternalInput")
    inorm_win = nc.dram_tensor("inorm_win", [NREL, 128, W49], F32, kind="ExternalInput")
    idx_t, s_t = {}, {}
    for r in range(NREL):
        for h in (0,):
            ns = nchunk[(r, h)] * CHUNK
            idx_t[(r, h)] = nc.dram_tensor(f"idx_{r}_{h}", [128, ns // 16], I16,
                                           kind="ExternalInput")
            s_t[(r, h)] = nc.dram_tensor(f"s_{r}_{h}", [128, 2 * ns], FP8,
                                         kind="ExternalInput")
    out_t = nc.dram_tensor("out", [3, 128, W49, OUT], F32, kind="ExternalOutput")

    # internal dram: per-relation table + bounce
    tables = [nc.dram_tensor(f"table_{r}", [N, HID], BF16) for r in range(NREL)]
    bounces = [nc.dram_tensor(f"bounce_{r}", [SH, HID], BF16) for r in range(NREL)]

    from contextlib import ExitStack
    with tile.TileContext(nc) as tc, ExitStack() as ctx:
        if True:
            p = lambda name, bufs, **kw: ctx.enter_context(tc.tile_pool(name=name, bufs=bufs, **kw))
            wts = p("wts", 1); hsp = p("hs", 1); accp = p("accp", 1)
            hfmp = p("hfm", 1); featp = p("feat", 3); idxp = p("idx", 2)
            edgep = p("edges", 6); sp = p("sp", 6); stagep = p("stage", 1)
            ostagep = p("ostage", 2)
            evp = p("ev", 3); lnp = p("ln", 8)
            psw = p("psw", 4, space="PSUM"); psm = p("psm", 2, space="PSUM")
            pst = p("pst", 2, space="PSUM")
            # ---- load params to sbuf
            ident = wts.tile([128, 128], F32)
            make_identity(nc, ident[:])
            eps_s = wts.tile([128, 1], F32)
            nc.vector.memset(eps_s[:], EPS)
            embW_s = wts.tile([IN_F, 3, EMB], F32)
            nc.sync.dma_start(out=embW_s[:], in_=embW[:, :, :].rearrange("t k m -> k t m"))
            embB_s = wts.tile([128, 3, EMB], F32)
            nc.sync.dma_start(out=embB_s[:], in_=embB[:, :, :])
            convW_s = wts.tile([EMB, NLAYERS * NREL, HID], F32)
            nc.sync.dma_start(out=convW_s[:],
                              in_=convW[:, :, :, :].rearrange("l r k m -> k (l r) m"))
            biasum_s = wts.tile([128, NLAYERS * 3, HID], F32)
            nc.sync.dma_start(out=biasum_s[:], in_=biasum[:, :, :])
            lng_s = wts.tile([128, 3, HID], F32)
            nc.sync.dma_start(out=lng_s[:], in_=lng[:, :, :])
            lnb_s = wts.tile([128, 3, HID], F32)
            nc.sync.dma_start(out=lnb_s[:], in_=lnb[:, :, :])
            clsW_s = wts.tile([HID, 3, OUT], F32)
            nc.sync.dma_start(out=clsW_s[:], in_=clsW[:, :, :].rearrange("t k m -> k t m"))
            clsb_s = wts.tile([128, 3, OUT], F32)
            nc.sync.dma_start(out=clsb_s[:], in_=clsb[:, :, :])
            inorm_s = wts.tile([128, NREL, W49], F32)
            nc.sync.dma_start(out=inorm_s[:], in_=inorm_win[:, :, :].rearrange("r p w -> p r w"))
            onorm_s = wts.tile([128, NREL, W49], F32)
            nc.sync.dma_start(out=onorm_s[:], in_=onorm_win[:, :, :].rearrange("r p w -> p r w"))

            h = [hsp.tile([128, W49, HID], F32, name=f"h{t}") for t in range(3)]
            acc = [accp.tile([128, W49, HID], F32, name=f"acc{t}") for t in range(3)]
            h_fm = hfmp.tile([HID, 128 * W49], F32)

            # ---- embed: h0[t] = feat_t @ embW[t] + embB[t]   (node-major out)
            for t in range(3):
                for c in range(W49):
                    fchunk = featp.tile([IN_F, 128], F32, tag="fc")
                    nc.sync.dma_start(out=fchunk[:],
                                      in_=feat_fm[t, :, c * 128:(c + 1) * 128])
                    pe = psm.tile([128, EMB], F32, tag="pmm", name="pe")
                    nc.tensor.matmul(pe[:], fchunk[:], embW_s[:, t, :],
                                     start=True, stop=True)
                    nc.vector.tensor_tensor(
                        out=h[t][:, c, :], in0=pe[:],
                        in1=embB_s[:, t, :],
                        op=mybir.AluOpType.add)

            # ---- layers
            def build_table(l, r):
                s_type = REL[r][0]
                # table shard rows = onorm * (h_fm.T chunks @ W)  (scale on evac)
                stg = stagep.tile([128, W49, HID], BF16, tag="tstage")
                for c in range(W49):
                    pt = psm.tile([128, HID], F32, tag="pmm", name="pt")
                    nc.tensor.matmul(pt[:], h_fm[:, c * 128:(c + 1) * 128],
                                     convW_s[:, l * NREL + r, :], start=True, stop=True)
                    nc.vector.tensor_scalar_mul(stg[:, c, :], pt[:],
                                                onorm_s[:, r, c:c + 1])
                # stage -> bounce dram (node-major rows)
                nc.sync.dma_start(
                    out=bounces[r][0:6144, :].rearrange("(a p) f -> p a f", p=128),
                    in_=stg[:, 0:48, :])
                nc.sync.dma_start(out=bounces[r][6144:SH, :], in_=stg[0:SH - 6144, 48, :])
                import os as _os
                if _os.environ.get("K_NO_CC"):
                    # debug: local copy instead of allgather (wrong results)
                    nc.sync.dma_start(out=tables[r][0:SH, :], in_=bounces[r][:, :])
                else:
                    nc.gpsimd.collective_compute(
                        "AllGather", mybir.AluOpType.bypass,
                        replica_groups=[list(range(NC))],
                        ins=[bounces[r][:, :].opt()], outs=[tables[r][:, :].opt()])

            def transpose_h(t):
                for c in range(W49):
                    ptr = pst.tile([HID, 128], F32, tag="ptr")
                    nc.tensor.transpose(out=ptr[:], in_=h[t][:, c, :], identity=ident[:])
                    nc.vector.tensor_copy(out=h_fm[:, c * 128:(c + 1) * 128], in_=ptr[:])

            def gather_reduce(l, r, hh, t):
                nbw = nbw_all[(r, hh)]
                nck = nchunk[(r, hh)]
                ns = nck * CHUNK
                idxs = idxp.tile([128, ns // 16], I16, tag="idx")
                nc.sync.dma_start(out=idxs[:], in_=idx_t[(r, hh)][:, :])
                # pair view: rows = node pairs (2p, 2p+1), 128 bf16 = 256B
                src_ap = tables[r][:, :].rearrange("(a b) f -> a (b f)", b=2)
                ebufs = {}
                sbufs = {}
                blk = 0
                pwin = None
                for w in range(W49):
                    for j in range(int(nbw[w])):
                        c = blk // BPC
                        if c not in ebufs:
                            eb = edgep.tile([128, BPC, 128], BF16, tag="eb")
                            nc.gpsimd.dma_gather(
                                eb[:], src_ap, idxs[:, c * 64:(c + 1) * 64],
                                CHUNK, CHUNK, 128)
                            sb = sp.tile([128, 2 * CHUNK], FP8, tag="sb")
                            nc.sync.dma_start(
                                out=sb[:],
                                in_=s_t[(r, hh)][:, 2 * c * CHUNK:2 * (c + 1) * CHUNK])
                            ebufs = {c: eb}
                            sbufs = {c: sb}
                        bb = blk % BPC
                        if j == 0:
                            pwin = psw.tile([128, HID], F32, tag="pw")
                        nc.tensor.matmul(
                            pwin[:], sbufs[c][:, bb * 256:bb * 256 + 128],
                            ebufs[c][:, bb, 0:HID],
                            start=(j == 0), stop=False)
                        nc.tensor.matmul(
                            pwin[:], sbufs[c][:, bb * 256 + 128:bb * 256 + 256],
                            ebufs[c][:, bb, HID:2 * HID],
                            start=False, stop=(j == int(nbw[w]) - 1))
                        blk += 1
                    # evac window w
                    tmp = evp.tile([128, HID], F32, tag="ev")
                    nc.vector.tensor_scalar_mul(tmp[:], pwin[:], inorm_s[:, r, w:w + 1])
                    nc.vector.tensor_tensor(out=acc[t][:, w, :], in0=acc[t][:, w, :],
                                            in1=tmp[:], op=mybir.AluOpType.add)

            def layer_norm(l, t):
                a = acc[t]
                nc.vector.tensor_tensor(
                    out=a[:], in0=a[:],
                    in1=biasum_s[:, l * 3 + t:l * 3 + t + 1, :].to_broadcast([128, W49, HID]),
                    op=mybir.AluOpType.add)
                ssum = lnp.tile([128, W49], F32, tag="ssum")
                nc.vector.tensor_reduce(out=ssum[:], in_=a[:],
                                        axis=mybir.AxisListType.X, op=mybir.AluOpType.add)
                x2 = stagep.tile([128, W49, HID], F32, tag="tstage", name="x2t")
                nc.vector.tensor_tensor(out=x2[:], in0=a[:], in1=a[:],
                                        op=mybir.AluOpType.mult)
                s2 = lnp.tile([128, W49], F32, tag="s2")
                nc.vector.tensor_reduce(out=s2[:], in_=x2[:],
                                        axis=mybir.AxisListType.X, op=mybir.AluOpType.add)
                m = lnp.tile([128, W49], F32, tag="m")
                nc.vector.tensor_scalar_mul(m[:], ssum[:], 1.0 / HID)
                msq = lnp.tile([128, W49], F32, tag="msq")
                nc.vector.tensor_tensor(out=msq[:], in0=m[:], in1=m[:],
                                        op=mybir.AluOpType.mult)
                v = lnp.tile([128, W49], F32, tag="v")
                nc.vector.tensor_scalar_mul(v[:], s2[:], 1.0 / HID)
                nc.vector.tensor_tensor(out=v[:], in0=v[:], in1=msq[:],
                                        op=mybir.AluOpType.subtract)
                sd = lnp.tile([128, W49], F32, tag="sd")
                nc.scalar.activation(sd[:], v[:], mybir.ActivationFunctionType.Sqrt,
                                     bias=eps_s[:])
                inv = lnp.tile([128, W49], F32, tag="inv")
                nc.vector.reciprocal(inv[:], sd[:])
                # normalize in place then write h[t]
                nc.vector.tensor_tensor(out=a[:], in0=a[:],
                                        in1=m[:].rearrange("p (w o) -> p w o", o=1).to_broadcast([128, W49, HID]),
                                        op=mybir.AluOpType.subtract)
                nc.vector.tensor_tensor(out=a[:], in0=a[:],
                                        in1=inv[:].rearrange("p (w o) -> p w o", o=1).to_broadcast([128, W49, HID]),
                                        op=mybir.AluOpType.mult)
                nc.vector.tensor_tensor(out=a[:], in0=a[:],
                                        in1=lng_s[:, t:t + 1, :].to_broadcast([128, W49, HID]),
                                        op=mybir.AluOpType.mult)
                nc.vector.tensor_tensor(out=a[:], in0=a[:],
                                        in1=lnb_s[:, t:t + 1, :].to_broadcast([128, W49, HID]),
                                        op=mybir.AluOpType.add)
                nc.vector.tensor_scalar_max(h[t][:], a[:], 0.0)

            for l in range(NLAYERS):
                t_order = [(l + i) % 3 for i in range(3)]
                rel_seq = [r for t in t_order for r in RELS_OF_T[t]]
                # tables: emit CC r_{i+1} before gathers of r_i (pipeline)
                built = set()
                done_src = set()

                def ensure_table(i):
                    if i < len(rel_seq) and i not in built:
                        r = rel_seq[i]
                        s = REL[r][0]
                        if s not in done_src:
                            transpose_h(s)
                            done_src.add(s)
                        build_table(l, r)
                        built.add(i)

                # h_fm holds one type at a time; group table builds by src type
                # to avoid re-transposing: emit all tables first, in src-type
                # groups, ordered so early-needed tables build first.
                order = sorted(range(len(rel_seq)), key=lambda i: (REL[rel_seq[i]][0]
                               not in {REL[rel_seq[0]][0]}, REL[rel_seq[i]][0], i))
                for i in order:
                    ensure_table(i)
                for t in t_order:
                    nc.vector.memset(acc[t][:], 0.0)
                for t in t_order:
                    for r in RELS_OF_T[t]:
                        gather_reduce(l, r, 0, t)
                    layer_norm(l, t)

            # ---- classifier
            for t in range(3):
                transpose_h(t)
                ostg = ostagep.tile([128, W49, OUT], F32, tag="ostage")
                for c in range(W49):
                    po = psm.tile([128, OUT], F32, tag="pmm", name="po")
                    nc.tensor.matmul(po[:], h_fm[:, c * 128:(c + 1) * 128],
                                     clsW_s[:, t, :], start=True, stop=True)
                    nc.vector.tensor_tensor(
                        out=ostg[:, c, :], in0=po[:],
                        in1=clsb_s[:, t, :],
                        op=mybir.AluOpType.add)
                nc.sync.dma_start(out=out_t[t, :, :, :], in_=ostg[:])
    nc.compile()
    return nc


# --------------------------------------------------------------------------- entry
def kernel(feat_assmpt, feat_rule, feat_non_assmpt, emb_W, emb_b, conv_W, conv_b,
           ln_g, ln_b, cls_W, cls_b, src, dst):
    feats = [np.asarray(feat_assmpt), np.asarray(feat_rule), np.asarray(feat_non_assmpt)]
    src = np.asarray(src)
    dst = np.asarray(dst)
    onorm, inorm, nbw_all, nchunk, per_core = _plan(src, dst)

    # bias sums per (l, t)
    conv_b = np.asarray(conv_b)
    biasum = np.zeros((NLAYERS, 3, HID), np.float32)
    for l in range(NLAYERS):
        for r, (_, t) in enumerate(REL):
            biasum[l, t] += conv_b[l, r]

    nc = _build(nbw_all, nchunk)

    in_maps = []
    for d in range(NC):
        sl = slice(d * SH, (d + 1) * SH)
        feat_fm = np.zeros((3, IN_F, 128 * W49), np.float32)
        for t in range(3):
            feat_fm[t, :, :SH] = feats[t][sl].T
        iw = np.zeros((NREL, 128, W49), np.float32)
        ow = np.zeros((NREL, 128, W49), np.float32)
        for r in range(NREL):
            pad = np.ones(128 * W49, np.float32)
            pad[:SH] = inorm[r, sl]
            iw[r] = pad.reshape(W49, 128).T
            pad2 = np.ones(128 * W49, np.float32)
            pad2[:SH] = onorm[r, sl]
            ow[r] = pad2.reshape(W49, 128).T
        m = {
            "feat_fm": feat_fm,
            "embW": np.asarray(emb_W, np.float32),
            "embB": np.broadcast_to(np.asarray(emb_b, np.float32), (128, 3, EMB)).copy(),
            "convW": np.asarray(conv_W, np.float32),
            "biasum": np.broadcast_to(biasum.reshape(NLAYERS * 3, HID), (128, NLAYERS * 3, HID)).copy(),
            "lng": np.broadcast_to(np.asarray(ln_g, np.float32), (128, 3, HID)).copy(),
            "lnb": np.broadcast_to(np.asarray(ln_b, np.float32), (128, 3, HID)).copy(),
            "clsW": np.asarray(cls_W, np.float32),
            "clsb": np.broadcast_to(np.asarray(cls_b, np.float32), (128, 3, OUT)).copy(),
            "onorm_win": ow,
            "inorm_win": iw,
        }
        for r in range(NREL):
            idx_rep, s_arr = per_core[d][(r, 0)]
            m["idx_" + str(r) + "_0"] = idx_rep
            m["s_" + str(r) + "_0"] = s_arr
        in_maps.append(m)

    import jax
    fn, in_names, out_names, out_avals, sharding = _make_runner(nc, NC)
    concat_in = [np.concatenate([m[n] for m in in_maps], axis=0) for n in in_names]
    zeros = [np.zeros((NC * a.shape[0], *a.shape[1:]), a.dtype) for a in out_avals]
    dev_in = [jax.device_put(x, sharding) for x in concat_in]
    dev_zeros = [jax.device_put(z, sharding) for z in zeros]
    outs = fn(*dev_in, *dev_zeros)
    res = np.asarray(outs[0]).reshape(NC, 3, 128, W49, OUT)

    # stash for timing (test.py)
    import time as _time

    def _timed(n=6):
        ts = []
        for _ in range(n):
            t0 = _time.perf_counter()
            r = fn(*dev_in, *dev_zeros)
            jax.block_until_ready(r)
            ts.append(_time.perf_counter() - t0)
        return ts
    globals()["_last_run"] = _timed

    full = np.zeros((3, N, OUT), np.float32)
    for d in range(NC):
        o = res[d]  # [3, 128, W49, OUT]
        for t in range(3):
            rows = o[t].transpose(1, 0, 2).reshape(128 * W49, OUT)[:SH]
            full[t, d * SH:(d + 1) * SH] = rows
    return full
